# revision 3
# baseline (speedup 1.0000x reference)
"""Involution layer (per-pixel dynamic 3x3 grouped filtering) on 8 trn2 cores.

Sharding: data-parallel over (batch, h-block): core i owns batch i//2,
h rows [(i%2)*64, (i%2)*64+64). Each core gets a 66-row x slab (1 halo row
each side, zero-padded at image edges); halos make the kernel purely local.

Per-core pipeline (fp32), processed in 4-row blocks (n = 4*128 = 512 pixels):
  - DMA 6 natural x rows [w,c]; PE-transpose to xT [c,(row,w+halo)] (ACT copies)
  - h = W1^T x (PE, K=c contracted in 2 chunks of 128)
  - hrelu = Relu(scale*h + bias) on ACT (BN affine folded in)
  - kerT[n,144] = hrelu_aug^T @ w2aug (PE, bias via ones row) -> kernel output
  - kerb_k[128,n] = w2xk_k^T @ hrelu_aug (PE): group-broadcast kernels,
    w2xk_k[d, m] = w2aug[d, k*16 + m%16]
  - involution: out[c,n] = sum_k xT_shift_k * kerb_k  (DVE mults+adds)
  - PE-transpose back to [w,c], DMA out.
"""
import os
import sys
import numpy as np

sys.path.insert(0, "/opt/trn_rl_repo")

import concourse.bass as bass
import concourse.bacc as bacc
import concourse.tile as tile
from concourse import mybir
from concourse.bass_utils import run_bass_kernel_spmd
from concourse.masks import make_identity

BN_EPS = 1e-3
B, H, W, C = 4, 128, 128, 256
G, K, RED = 16, 3, 4
K2 = K * K
CR = C // RED          # 64
E = K2 * G             # 144
ROWS = 64              # output rows per core
SLAB = ROWS + 2        # input rows incl halo
R = 4                  # rows per block
NBLK = ROWS // R
N = R * W              # 512 pixels per block
F32 = mybir.dt.float32

_CACHE = {}


def _build_program():
    nc = bacc.Bacc("TRN2", target_bir_lowering=False, debug=False, num_devices=8)

    x_dram = nc.dram_tensor("x_slab", [SLAB, W, C], F32, kind="ExternalInput").ap()
    w1_dram = nc.dram_tensor("w1c", [2, 128, CR], F32, kind="ExternalInput").ap()
    scale_dram = nc.dram_tensor("actscale", [CR, 1], F32, kind="ExternalInput").ap()
    bias_dram = nc.dram_tensor("actbias", [CR, 1], F32, kind="ExternalInput").ap()
    w2aug_dram = nc.dram_tensor("w2aug", [CR + 1, E], F32, kind="ExternalInput").ap()
    w2xk_dram = nc.dram_tensor("w2xk", [K2, CR + 1, 128], F32, kind="ExternalInput").ap()
    out_dram = nc.dram_tensor("out_slab", [ROWS, W, C], F32, kind="ExternalOutput").ap()
    ker_dram = nc.dram_tensor("ker_slab", [ROWS, W, E], F32, kind="ExternalOutput").ap()

    with tile.TileContext(nc) as tc:
        _kernel_body(tc, x_dram, w1_dram, scale_dram, bias_dram, w2aug_dram,
                     w2xk_dram, out_dram, ker_dram)
    nc.compile()
    return nc


def _kernel_body(tc, x_dram, w1_dram, scale_dram, bias_dram, w2aug_dram,
                 w2xk_dram, out_dram, ker_dram):
    nc = tc.nc
    from contextlib import ExitStack
    ctx = ExitStack()
    with ctx:
        consts = ctx.enter_context(tc.tile_pool(name="consts", bufs=1))
        xnat_p = ctx.enter_context(tc.tile_pool(name="xnat", bufs=12))
        stage_p = ctx.enter_context(tc.tile_pool(name="stage", bufs=3))
        dve_p = ctx.enter_context(tc.tile_pool(name="dve", bufs=2))
        ps_tp = ctx.enter_context(tc.tile_pool(name="ps_tp", bufs=3, space="PSUM"))
        ps_ho = ctx.enter_context(tc.tile_pool(name="ps_ho", bufs=1, space="PSUM"))
        ps_kb = ctx.enter_context(tc.tile_pool(name="ps_kb", bufs=3, space="PSUM"))

        # ---- constants ----
        w1_sb = consts.tile([128, 2, CR], F32)
        nc.sync.dma_start(out=w1_sb, in_=w1_dram.rearrange("j p d -> p j d"))
        scale_sb = consts.tile([CR, 1], F32)
        nc.sync.dma_start(out=scale_sb, in_=scale_dram)
        bias_sb = consts.tile([CR, 1], F32)
        nc.sync.dma_start(out=bias_sb, in_=bias_dram)
        w2aug_sb = consts.tile([CR + 1, E], F32)
        nc.sync.dma_start(out=w2aug_sb, in_=w2aug_dram)
        w2xk_sb = consts.tile([CR + 1, K2, 128], F32)
        nc.sync.dma_start(out=w2xk_sb, in_=w2xk_dram.rearrange("k d m -> d k m"))
        ident = consts.tile([128, 128], F32)
        make_identity(nc, ident)

        # xT slabs (even/odd), halo cols preset to zero once
        xT_slabs = []
        for p in range(2):
            t = consts.tile([128, 2, R + 2, W + 2], F32, name=f"xT{p}", tag=f"xT{p}")
            nc.vector.memset(t[:, :, :, 0:1], 0.0)
            nc.vector.memset(t[:, :, :, W + 1:W + 2], 0.0)
            xT_slabs.append(t)
        # hrelu_aug (even/odd), ones row preset
        hrelu_augs = []
        for p in range(2):
            t = consts.tile([CR + 1, N], F32, name=f"hrelu{p}", tag=f"hrelu{p}")
            nc.vector.memset(t[CR:CR + 1, :], 1.0)
            hrelu_augs.append(t)

        for blk in range(NBLK):
            r0 = R * blk
            xT = xT_slabs[blk % 2]
            hrelu_aug = hrelu_augs[blk % 2]

            # ---- load + transpose x rows ----
            for s in range(R + 2):
                xnat = xnat_p.tile([W, C], F32, tag="xnat")
                nc.sync.dma_start(out=xnat, in_=x_dram[r0 + s])
                for j in range(2):
                    tp = ps_tp.tile([128, 128], F32, tag="tpkt")
                    nc.tensor.transpose(tp, xnat[:, j * 128:(j + 1) * 128], ident)
                    nc.scalar.copy(xT[:, j, s, 1:W + 1], tp)

            # ---- kernel generation ----
            h_ps = ps_ho.tile([CR, N], F32, tag="h_out")
            for j in range(2):
                nc.tensor.matmul(h_ps, lhsT=w1_sb[:, j, :],
                                 rhs=xT[:, j, 1:R + 1, 1:W + 1],
                                 start=(j == 0), stop=(j == 1))
            nc.scalar.activation(hrelu_aug[0:CR, :], h_ps,
                                 mybir.ActivationFunctionType.Relu,
                                 bias=bias_sb, scale=scale_sb)

            ksb = stage_p.tile([128, R, E], F32, tag="ksb")
            for half in range(2):
                kt = ps_tp.tile([128, 2, E], F32, tag="tpkt")
                for i2 in range(2):
                    i = half * 2 + i2
                    nc.tensor.matmul(kt[:, i2, :],
                                     lhsT=hrelu_aug[:, i * 128:(i + 1) * 128],
                                     rhs=w2aug_sb, start=True, stop=True)
                nc.scalar.copy(ksb[:, half * 2:half * 2 + 2, :], kt)
            nc.sync.dma_start(out=ker_dram[r0:r0 + R].rearrange("r w e -> w r e"),
                              in_=ksb)

            # ---- group-broadcast kernels + involution ----
            accs = []
            for j in range(2):
                accs.append(dve_p.tile([128, R, W], F32, tag=f"acc{j}",
                                       name=f"acc{j}"))
            for k in range(K2):
                di, dj = k // 3, k % 3
                kb = ps_kb.tile([128, R, W], F32, tag="kerb")
                nc.tensor.matmul(kb, lhsT=w2xk_sb[:, k, :], rhs=hrelu_aug,
                                 start=True, stop=True)
                for j in range(2):
                    x_term = xT[:, j, di:di + R, dj:dj + W]
                    if k == 0:
                        nc.vector.tensor_mul(accs[j], x_term, kb)
                    else:
                        tmp = dve_p.tile([128, R, W], F32, tag="tmp")
                        nc.vector.tensor_mul(tmp, x_term, kb)
                        nc.vector.tensor_add(accs[j], accs[j], tmp)

            # ---- transpose out back to [w, c] and store ----
            ops = ps_ho.tile([128, R, C], F32, tag="h_out")
            for j in range(2):
                for i in range(R):
                    nc.tensor.transpose(ops[:, i, j * 128:(j + 1) * 128],
                                        accs[j][:, i, :], ident)
            osb = stage_p.tile([128, R, C], F32, tag="osb")
            nc.scalar.copy(osb, ops)
            nc.sync.dma_start(out=out_dram[r0:r0 + R].rearrange("r w c -> w r c"),
                              in_=osb)


def _host_prep(w1, b1, gamma, beta, bn_mean, bn_var, w2, b2):
    s = (gamma / np.sqrt(bn_var + BN_EPS)).astype(np.float32)
    actscale = np.ascontiguousarray(s[:, None])
    actbias = np.ascontiguousarray((((b1 - bn_mean) * s) + beta)[:, None]).astype(np.float32)
    w1c = np.ascontiguousarray(w1.reshape(2, 128, CR)).astype(np.float32)
    w2aug = np.vstack([w2, b2[None]]).astype(np.float32)
    m = np.arange(128)
    w2xk = np.ascontiguousarray(
        np.stack([w2aug[:, k * G + (m % G)] for k in range(K2)])).astype(np.float32)
    return actscale, actbias, w1c, w2aug, w2xk


def kernel(x, w1, b1, gamma, beta, bn_mean, bn_var, w2, b2, _profile=None):
    x = np.asarray(x, np.float32)
    actscale, actbias, w1c, w2aug, w2xk = _host_prep(
        np.asarray(w1, np.float32), np.asarray(b1, np.float32),
        np.asarray(gamma, np.float32), np.asarray(beta, np.float32),
        np.asarray(bn_mean, np.float32), np.asarray(bn_var, np.float32),
        np.asarray(w2, np.float32), np.asarray(b2, np.float32))

    if "nc" not in _CACHE:
        _CACHE["nc"] = _build_program()
    nc = _CACHE["nc"]

    in_maps = []
    for i in range(8):
        bi, h0 = i // 2, (i % 2) * ROWS
        xs = np.zeros((SLAB, W, C), np.float32)
        xs[1:1 + ROWS] = x[bi, h0:h0 + ROWS]
        if h0 > 0:
            xs[0] = x[bi, h0 - 1]
        if h0 + ROWS < H:
            xs[1 + ROWS] = x[bi, h0 + ROWS]
        in_maps.append({
            "x_slab": xs, "w1c": w1c, "actscale": actscale, "actbias": actbias,
            "w2aug": w2aug, "w2xk": w2xk,
        })

    kwargs = dict(_profile) if _profile else {}
    res = run_bass_kernel_spmd(nc, in_maps, list(range(8)), **kwargs)
    if _profile is not None:
        _CACHE["last_result"] = res

    out = np.empty((B, H, W, C), np.float32)
    ker = np.empty((B, H, W, E), np.float32)
    for i in range(8):
        bi, h0 = i // 2, (i % 2) * ROWS
        out[bi, h0:h0 + ROWS] = res.results[i]["out_slab"]
        ker[bi, h0:h0 + ROWS] = res.results[i]["ker_slab"]
    return out, ker.reshape(B, H, W, K2, 1, G)
